# revision 1
# baseline (speedup 1.0000x reference)
"""GQA attention prefill (B=4, S=2048, D=4096, 32 q-heads / 8 kv-heads, rotary,
causal) on 8 TRN2 NeuronCores.

Sharding: token-parallel ("zigzag" sequence split) — core c handles batch
c//2 and two 512-token chunks of its sequence: chunks {0,3} for even cores,
{1,2} for odd cores (balances the causal triangle). Each core computes full
Q projection for its tokens, K/V for the whole prefix, attention for all 32
heads over its tokens, and the output projection for its tokens. No
inter-core communication: outputs are disjoint token slices, gathered on
host.

One SPMD Bass program for all 8 cores; per-core divergence (chunk positions,
causal masks, padded key-blocks) is entirely data-driven via per-core input
arrays.

Precision: Q/K/V projections run bf16 x bf16 -> f32 PSUM; attention scores,
softmax, AV and the output projection run float32r (full-rate fp32).

Device layout conventions:
  - activations for QK^T are kept transposed: [head_dim (partitions), tokens]
  - rotary pairs are de-interleaved (even dims -> partitions 0-63, odd ->
    64-127) via a host-side permutation of the qw/kw rows; scores are
    invariant to this shared permutation.
  - attention runs in "scores-transposed" orientation: ST[key, query] =
    kT.T @ qT, softmax over the partition (key) axis with the denominator
    computed by a ones-vector matmul; no max-subtraction (scores are O(1):
    unit-variance inputs, 1/sqrt(head) scaling).
  - DMA routing: large batched loads on nc.sync (HWDGE, 625ns fixed cost on
    a shared device); high-count weight/output streams on nc.gpsimd (SWDGE,
    runs on the otherwise-idle Pool engine).
"""

import numpy as np
import ml_dtypes

import concourse.bacc as bacc
import concourse.bass as bass
import concourse.tile as tile
from concourse import library_config, mybir
from concourse.bass_utils import run_bass_kernel_spmd

F32 = mybir.dt.float32
F32R = mybir.dt.float32r
BF16 = mybir.dt.bfloat16
EXP = mybir.ActivationFunctionType.Exp
COPY = mybir.ActivationFunctionType.Copy
ADD = mybir.AluOpType.add
MULT = mybir.AluOpType.mult

B, S, D = 4, 2048, 4096
QH, KVH, HEAD = 32, 8, 128
P = 128
CH = 512                # token chunk (= query tile)
NCH = S // CH           # 4 chunks per sequence
ND = D // P             # 32 d-tiles
NCORES = 8
NKB = (8, 16)           # key-blocks per query slot (padded, uniform)
SCALE = 1.0 / np.sqrt(HEAD)
BF = ml_dtypes.bfloat16

_CACHE = {}


def _build():
    nc = bacc.Bacc("TRN2", target_bir_lowering=False, debug=False, num_devices=NCORES)

    # ---- per-core external inputs ----
    # strips layouts are [.., dp, dt, t] so each SBUF partition row is one
    # contiguous DRAM run (dt*t elements)
    own = nc.dram_tensor("own_strips", [2, P, ND, CH], BF16, kind="ExternalInput")
    pref = nc.dram_tensor("in_strips", [NCH, P, ND, CH], BF16, kind="ExternalInput")
    # weight layouts [head, dp, dt, fp]: one contiguous 2D load per head
    qwT = nc.dram_tensor("qwT", [QH, P, ND, P], BF16, kind="ExternalInput")
    kwT = nc.dram_tensor("kwT", [KVH, P, ND, P], BF16, kind="ExternalInput")
    vwT = nc.dram_tensor("vwT", [2, ND, P, 512], BF16, kind="ExternalInput")
    owT = nc.dram_tensor("owT", [8, ND, P, 512], BF16, kind="ExternalInput")
    cos_own = nc.dram_tensor("cos_own", [64, 2, CH], F32, kind="ExternalInput")
    sin_own = nc.dram_tensor("sin_own", [64, 2, CH], F32, kind="ExternalInput")
    cos_all = nc.dram_tensor("cos_all", [64, S], F32, kind="ExternalInput")
    sin_all = nc.dram_tensor("sin_all", [64, S], F32, kind="ExternalInput")
    qbT = nc.dram_tensor("qbT", [P, QH], F32, kind="ExternalInput")
    kbT = nc.dram_tensor("kbT", [P, KVH], F32, kind="ExternalInput")
    vb = nc.dram_tensor("vb", [KVH * HEAD], F32, kind="ExternalInput")
    ob = nc.dram_tensor("ob", [D], F32, kind="ExternalInput")
    masks = nc.dram_tensor("masks", [2, 8, P, CH], BF16, kind="ExternalInput")
    ones = nc.dram_tensor("ones", [P], BF16, kind="ExternalInput")

    # ---- internal DRAM ----
    qT_i = nc.dram_tensor("qT_i", [2, QH, P, CH], F32R)
    kT_i = nc.dram_tensor("kT_i", [KVH, P, S], F32R)
    v_i = nc.dram_tensor("v_i", [2, 16, P, 512], BF16)   # [hs, kb, kj, j*128+hd]
    oT_i = nc.dram_tensor("oT_i", [2, QH, P, CH], BF16)

    out = nc.dram_tensor("out", [8, P, D], F32, kind="ExternalOutput")

    with tile.TileContext(nc) as tc:
        nc.gpsimd.load_library(library_config.lib)
        with (
            tc.tile_pool(name="const", bufs=1) as const,
            tc.tile_pool(name="ev", bufs=4) as evpool,
            tc.tile_pool(name="rt", bufs=4) as rtpool,
            tc.tile_pool(name="ps", bufs=8, space="PSUM") as pspool,
        ):
            kbT_e = const.tile([64, KVH], F32, tag="kbte")
            kbT_o = const.tile([64, KVH], F32, tag="kbto")
            nc.sync.dma_start(out=kbT_e[:], in_=kbT[0:64, :])
            nc.sync.dma_start(out=kbT_o[:], in_=kbT[64:P, :])
            ones_col = const.tile([P, 1], BF16, tag="oc")
            nc.sync.dma_start(out=ones_col[:], in_=ones.ap()[:, None])

            def rotary_evict(ps, dst, cos_ap, sin_ap, be, bo):
                """dst[0:64]=(pe+be)*cos-(po+bo)*sin; dst[64:128]=(pe+be)*sin+(po+bo)*cos"""
                pe, po = ps[0:64, :], ps[64:128, :]
                t1 = rtpool.tile([64, CH], F32, tag="rt", name="t1")
                t2 = rtpool.tile([64, CH], F32, tag="rt", name="t2")
                nc.vector.scalar_tensor_tensor(t1[:], pe, be, cos_ap, ADD, MULT)
                nc.vector.scalar_tensor_tensor(t2[:], po, bo, sin_ap, ADD, MULT)
                nc.vector.tensor_sub(dst[0:64, :], t1[:], t2[:])
                t3 = rtpool.tile([64, CH], F32, tag="rt", name="t3")
                t4 = rtpool.tile([64, CH], F32, tag="rt", name="t4")
                nc.vector.scalar_tensor_tensor(t3[:], pe, be, sin_ap, ADD, MULT)
                nc.vector.scalar_tensor_tensor(t4[:], po, bo, cos_ap, ADD, MULT)
                nc.vector.tensor_add(dst[64:128, :], t3[:], t4[:])

            # ====== P1 + P0 share the strip / weight pools ======
            w_cm = tc.tile_pool(name="w", bufs=2)
            wpool = w_cm.__enter__()
            p01_cm = tc.tile_pool(name="strip", bufs=3)
            strip_pool = p01_cm.__enter__()

            # ============ P1: K/V projection over full prefix ============
            with tc.tile_pool(name="p1c", bufs=1) as p1c:
                cos_all_sb = p1c.tile([64, S], F32, tag="cosa")
                sin_all_sb = p1c.tile([64, S], F32, tag="sina")
                nc.sync.dma_start(out=cos_all_sb[:], in_=cos_all[:])
                nc.sync.dma_start(out=sin_all_sb[:], in_=sin_all[:])
                vb_sb = p1c.tile([P, KVH * HEAD], F32, tag="vb")
                nc.sync.dma_start(
                    out=vb_sb[:], in_=vb.ap()[None, :].partition_broadcast(P)
                )
                with tc.tile_pool(name="wb", bufs=6) as wbpool:
                    for pr in range(2):
                        strips = []
                        for i in range(2):
                            st = strip_pool.tile(
                                [P, ND, CH], BF16, tag="strip", name=f"strip{pr}_{i}"
                            )
                            nc.sync.dma_start(out=st[:], in_=pref[2 * pr + i])
                            strips.append(st)
                        # K-pass (weight-stationary, out = kT [f, t])
                        for kv in range(KVH):
                            w = wpool.tile([P, ND, P], BF16, tag="w", name=f"kw{kv}")
                            nc.gpsimd.dma_start(out=w[:], in_=kwT[kv])
                            for ts in range(2):
                                tg = 2 * pr + ts
                                ps = pspool.tile([P, CH], F32, tag="ps", name="ps_k")
                                for dt in range(ND):
                                    nc.tensor.matmul(
                                        ps[:], lhsT=w[:, dt, :], rhs=strips[ts][:, dt, :],
                                        start=(dt == 0), stop=(dt == ND - 1),
                                    )
                                krot = evpool.tile([P, CH], F32R, tag="ev", name="krot")
                                rotary_evict(
                                    ps, krot,
                                    cos_all_sb[:, tg * CH : (tg + 1) * CH],
                                    sin_all_sb[:, tg * CH : (tg + 1) * CH],
                                    kbT_e[:, kv : kv + 1], kbT_o[:, kv : kv + 1],
                                )
                                nc.sync.dma_start(
                                    out=kT_i[kv, :, tg * CH : (tg + 1) * CH], in_=krot[:]
                                )
                        # V-pass (input-stationary, out = v [t, hd])
                        for hs in range(2):
                            psv = [
                                pspool.tile([P, 512], F32, tag="ps", name=f"psv{i}")
                                for i in range(8)
                            ]
                            for dt in range(ND):
                                vw = wbpool.tile([P, 512], BF16, tag="wb", name="vw")
                                nc.gpsimd.dma_start(out=vw[:], in_=vwT[hs, dt])
                                for ts in range(2):
                                    for tt in range(4):
                                        nc.tensor.matmul(
                                            psv[ts * 4 + tt][:],
                                            lhsT=strips[ts][:, dt, tt * P : (tt + 1) * P],
                                            rhs=vw[:],
                                            start=(dt == 0), stop=(dt == ND - 1),
                                        )
                            for ts in range(2):
                                for tt in range(4):
                                    kb = (2 * pr + ts) * 4 + tt
                                    vsb = evpool.tile([P, 512], BF16, tag="evb", name="vsb")
                                    nc.vector.tensor_add(
                                        vsb[:], psv[ts * 4 + tt][:],
                                        vb_sb[:, hs * 512 : (hs + 1) * 512],
                                    )
                                    nc.sync.dma_start(out=v_i[hs, kb], in_=vsb[:])

            p01_cm.__exit__(None, None, None)
            kv_cm = tc.tile_pool(name="kvS", bufs=3)
            kvpool = kv_cm.__enter__()
            qt_cm = tc.tile_pool(name="qtS", bufs=2)
            qtpool = qt_cm.__enter__()
            pt_cm = tc.tile_pool(name="ptS", bufs=4)
            ptpool = pt_cm.__enter__()
            r_cm = tc.tile_pool(name="rS", bufs=2)
            rpool = r_cm.__enter__()
            p0s_cm = tc.tile_pool(name="p0strip", bufs=2)
            p0strip_pool = p0s_cm.__enter__()

            # ============ P0: Q projection + rotary -> qT_i ============
            # Slot 0 is emitted eagerly; slot 1 is a generator woven into
            # P3-slot0's key-block loop so its dense matmuls fill the PE
            # stream while the attention pipeline waits on exp/mask.
            p0c_cm = tc.tile_pool(name="p0c", bufs=1)
            p0c = p0c_cm.__enter__()
            cos_own_sb = p0c.tile([64, 2, CH], F32, tag="coso")
            sin_own_sb = p0c.tile([64, 2, CH], F32, tag="sino")
            nc.sync.dma_start(out=cos_own_sb[:], in_=cos_own[:])
            nc.sync.dma_start(out=sin_own_sb[:], in_=sin_own[:])
            qbT_e = p0c.tile([64, QH], F32, tag="qbte")
            qbT_o = p0c.tile([64, QH], F32, tag="qbto")
            nc.sync.dma_start(out=qbT_e[:], in_=qbT[0:64, :])
            nc.sync.dma_start(out=qbT_o[:], in_=qbT[64:P, :])

            def p0_heads(sl, group):
                """Emit Q-proj for one slot; yield after each `group` matmuls."""
                st = p0strip_pool.tile([P, ND, CH], BF16, tag="p0strip", name=f"ostrip{sl}")
                nc.sync.dma_start(out=st[:], in_=own[sl])
                for h in range(QH):
                    w = wpool.tile([P, ND, P], BF16, tag="w", name=f"qw{sl}_{h}")
                    nc.gpsimd.dma_start(out=w[:], in_=qwT[h])
                    ps = pspool.tile([P, CH], F32, tag="ps", name="ps_q")
                    for dtg in range(ND // group):
                        for k in range(group):
                            dt = dtg * group + k
                            nc.tensor.matmul(
                                ps[:], lhsT=w[:, dt, :], rhs=st[:, dt, :],
                                start=(dt == 0), stop=(dt == ND - 1),
                            )
                        yield
                    qrot = evpool.tile([P, CH], F32R, tag="ev", name="qrot")
                    rotary_evict(
                        ps, qrot,
                        cos_own_sb[:, sl, :], sin_own_sb[:, sl, :],
                        qbT_e[:, h : h + 1], qbT_o[:, h : h + 1],
                    )
                    nc.sync.dma_start(out=qT_i[sl, h], in_=qrot[:])

            for _ in p0_heads(0, ND):
                pass

            def p4_half(hf, yield_every, otr, wb4pool, ob_sb):
                """Emit o-proj for token-slot half `hf` (ttiles 4hf..4hf+3)."""
                for hq in range(0, QH, 8):
                    nc.sync.dma_start(
                        out=otr[:, hq : hq + 8, :],
                        in_=oT_i[hf, hq : hq + 8].rearrange("h p t -> p h t"),
                    )
                for e in range(8):
                    ps4 = [
                        pspool.tile([P, 512], F32, tag="ps", name=f"ps4_{i}")
                        for i in range(4)
                    ]
                    cnt = 0
                    for f4 in range(ND // 4):
                        ow = wb4pool.tile([P, 4, 512], BF16, tag="wb4", name="ow")
                        nc.gpsimd.dma_start(
                            out=ow[:],
                            in_=owT[e, 4 * f4 : 4 * f4 + 4].rearrange("d p j -> p d j"),
                        )
                        for df in range(4):
                            ft = 4 * f4 + df
                            for tsub in range(4):
                                nc.tensor.matmul(
                                    ps4[tsub][:],
                                    lhsT=otr[:, ft, tsub * P : (tsub + 1) * P],
                                    rhs=ow[:, df, :],
                                    start=(ft == 0), stop=(ft == ND - 1),
                                )
                                cnt += 1
                                if cnt % yield_every == 0:
                                    yield
                    for tsub in range(4):
                        osb = evpool.tile([P, 512], F32, tag="ev4", name="osb4")
                        nc.vector.tensor_add(
                            osb[:], ps4[tsub][:], ob_sb[:, e * 512 : (e + 1) * 512]
                        )
                        nc.sync.dma_start(
                            out=out[hf * 4 + tsub, :, e * 512 : (e + 1) * 512],
                            in_=osb[:],
                        )

            def attn_slot(sl, feeder):
                n_kb = NKB[sl]
                with (
                    tc.tile_pool(name=f"mask{sl}", bufs=1) as mpool,
                    tc.tile_pool(name=f"v4{sl}", bufs=1) as v4pool,
                ):
                    msk = mpool.tile([P, 8, CH], BF16, tag="mask", name="msk")
                    nc.sync.dma_start(
                        out=msk[:], in_=masks[sl].rearrange("m k q -> k m q")
                    )
                    for hs in range(2):
                        v4 = v4pool.tile([P, n_kb, 512], BF16, tag="v4", name="v4")
                        nc.sync.dma_start(
                            out=v4[:, 0:n_kb, :],
                            in_=v_i[hs, 0:n_kb].rearrange("b p j -> p b j"),
                        )
                        for j in range(4):
                            kv = 4 * hs + j
                            kt = kvpool.tile([P, n_kb * P], F32R, tag="kt", name="kt")
                            nc.sync.dma_start(
                                out=kt[:, 0 : n_kb * P], in_=kT_i[kv, :, 0 : n_kb * P]
                            )
                            qt4 = qtpool.tile([P, 4, CH], F32R, tag="qt", name="qt4")
                            nc.sync.dma_start(
                                out=qt4[:],
                                in_=qT_i[sl, kv :: KVH].rearrange("g p t -> p g t"),
                            )
                            for g in range(4):
                                h = kv + KVH * g
                                oT_ps = pspool.tile([P, CH], F32, tag="ps", name="oT_ps")
                                sums_ps = pspool.tile([P, CH], F32, tag="ps", name="sums_ps")
                                for kb in range(n_kb):
                                    st_ps = pspool.tile([P, CH], F32, tag="ps", name="st_ps")
                                    nc.tensor.matmul(
                                        st_ps[:],
                                        lhsT=kt[:, kb * P : (kb + 1) * P],
                                        rhs=qt4[:, g, :], start=True, stop=True,
                                    )
                                    pt = ptpool.tile([P, CH], BF16, tag="pt", name="pt")
                                    nc.scalar.activation(pt[:], st_ps[:], EXP, scale=SCALE)
                                    if sl == 0 or kb >= 8:
                                        mi = kb if sl == 0 else kb - 8
                                        nc.vector.tensor_mul(pt[:], pt[:], msk[:, mi, :])
                                    if kb % 2 == 0:
                                        pt_prev = pt
                                    else:
                                        pp = ptpool.tile([P, CH], BF16, tag="ptp", name="pp")
                                        nc.vector.tensor_add(pp[:], pt_prev[:], pt[:])
                                        if kb % 4 == 1:
                                            pp_prev = pp
                                        else:
                                            pq = ptpool.tile([P, CH], BF16, tag="ptq", name="pq")
                                            nc.vector.tensor_add(pq[:], pp_prev[:], pp[:])
                                            nc.tensor.matmul(
                                                sums_ps[0:1, :], lhsT=ones_col[:], rhs=pq[:],
                                                start=(kb == 3), stop=(kb == n_kb - 1),
                                            )
                                    nc.tensor.matmul(
                                        oT_ps[:],
                                        lhsT=v4[:, kb, j * P : (j + 1) * P],
                                        rhs=pt[:],
                                        start=(kb == 0), stop=(kb == n_kb - 1),
                                    )
                                    if feeder is not None:
                                        next(feeder, None)
                                rsb = rpool.tile([1, CH], F32R, tag="r", name="rsb")
                                with nc.allow_low_precision(reason="f32r softmax denom"):
                                    nc.vector.reciprocal(rsb[:], sums_ps[0:1, :])
                                rb_bc = ptpool.tile([P, CH], F32R, tag="ptr", name="rb_bc")
                                nc.gpsimd.partition_broadcast(rb_bc[:], rsb[:])
                                osb = evpool.tile([P, CH], BF16, tag="evb", name="osb")
                                nc.vector.tensor_mul(osb[:], oT_ps[:], rb_bc[:])
                                nc.sync.dma_start(out=oT_i[sl, h], in_=osb[:])
                    if feeder is not None:
                        for _ in feeder:
                            pass

            # ==== P3 slot 0 woven with P0 slot 1 ====
            attn_slot(0, p0_heads(1, 4))
            p0c_cm.__exit__(None, None, None)
            p0s_cm.__exit__(None, None, None)

            # ==== P3 slot 1 woven with P4 half 0; then P4 half 1 ====
            with tc.tile_pool(name="obp", bufs=1) as obp:
                ob_sb = obp.tile([P, D], F32, tag="ob")
                nc.sync.dma_start(
                    out=ob_sb[:], in_=ob.ap()[None, :].partition_broadcast(P)
                )
                with (
                    tc.tile_pool(name="p4a", bufs=1) as p4a,
                    tc.tile_pool(name="wb4a", bufs=2) as wb4a,
                ):
                    otr0 = p4a.tile([P, QH, CH], BF16, tag="ot0")
                    attn_slot(1, p4_half(0, 2, otr0, wb4a, ob_sb))
                with (
                    tc.tile_pool(name="p4b", bufs=1) as p4b,
                    tc.tile_pool(name="wb4b", bufs=3) as wb4b,
                ):
                    otr1 = p4b.tile([P, QH, CH], BF16, tag="ot1")
                    for _ in p4_half(1, 1 << 30, otr1, wb4b, ob_sb):
                        pass
            r_cm.__exit__(None, None, None)
            pt_cm.__exit__(None, None, None)
            qt_cm.__exit__(None, None, None)
            kv_cm.__exit__(None, None, None)
            w_cm.__exit__(None, None, None)

    nc.compile()
    return nc


def _get_nc():
    if "nc" not in _CACHE:
        _CACHE["nc"] = _build()
    return _CACHE["nc"]


_PERM = np.concatenate([np.arange(0, P, 2), np.arange(1, P, 2)])


def _prep_shared(qw_w, qw_b, kw_w, kw_b, vw_w, vw_b, ow_w, ow_b, fc, fs):
    f32 = np.float32
    c = np.ascontiguousarray
    # [h, dp, dt, fp] = w[h*128 + perm[fp], dt*128 + dp]
    qq = qw_w.reshape(QH, P, D)[:, _PERM, :]                      # [h, fp, d]
    qwT = c(qq.reshape(QH, P, ND, P).transpose(0, 3, 2, 1).astype(BF))
    kk = kw_w.reshape(KVH, P, D)[:, _PERM, :]
    kwT = c(kk.reshape(KVH, P, ND, P).transpose(0, 3, 2, 1).astype(BF))
    # [hs, dt, dp, j] = vw[hs*512 + j, dt*128 + dp]
    vwT = c(vw_w.reshape(2, 512, ND, P).transpose(0, 2, 3, 1).astype(BF))
    # [es, ft, fp, j] = ow[es*512 + j, ft*128 + fp]
    owT = c(ow_w.reshape(8, 512, ND, P).transpose(0, 2, 3, 1).astype(BF))
    cos_all = c(fc.T.astype(f32))  # [64, S]
    sin_all = c(fs.T.astype(f32))
    qbT = c(qw_b.reshape(QH, P)[:, _PERM].T.astype(f32))
    kbT = c(kw_b.reshape(KVH, P)[:, _PERM].T.astype(f32))
    return dict(
        qwT=qwT, kwT=kwT, vwT=vwT, owT=owT,
        cos_all=cos_all, sin_all=sin_all, qbT=qbT, kbT=kbT,
        vb=c(vw_b.astype(f32)), ob=c(ow_b.astype(f32)),
    )


def _masks_for(chunks):
    m = np.zeros((2, 8, P, CH), BF)
    kp = np.arange(P)[:, None]
    qi = np.arange(CH)[None, :]
    for sl in range(2):
        q0 = chunks[sl] * CH
        for mi in range(8):
            kb = mi if sl == 0 else mi + 8
            m[sl, mi] = (kb * P + kp <= q0 + qi).astype(BF)
    return m


def _core_chunks(core):
    b, par = core // 2, core % 2
    return b, ((0, 3) if par == 0 else (1, 2))


def _make_in_maps(inputs):
    """inputs: dict with the reference's setup_inputs() keys (numpy)."""
    g = lambda k: np.asarray(inputs[k])
    shared = _prep_shared(
        g("qw_w"), g("qw_b"), g("kw_w"), g("kw_b"), g("vw_w"), g("vw_b"),
        g("ow_w"), g("ow_b"), g("freqs_cos"), g("freqs_sin"),
    )
    input = g("input")
    in_maps = []
    for core in range(NCORES):
        b, chunks = _core_chunks(core)
        x = input[b].astype(np.float32)  # [S, D]
        # [s, dp, dt, t] = x[s*512 + t, dt*128 + dp]
        strips = np.ascontiguousarray(
            x.reshape(NCH, CH, ND, P).transpose(0, 3, 2, 1).astype(BF)
        )
        own = np.ascontiguousarray(strips[list(chunks)])
        cos_own = np.ascontiguousarray(
            np.stack([shared["cos_all"][:, c * CH : (c + 1) * CH] for c in chunks], 1)
        )
        sin_own = np.ascontiguousarray(
            np.stack([shared["sin_all"][:, c * CH : (c + 1) * CH] for c in chunks], 1)
        )
        m = dict(shared)
        m.update(
            ones=np.ones(P, BF),
            own_strips=own, in_strips=strips,
            cos_own=cos_own, sin_own=sin_own, masks=_masks_for(chunks),
        )
        in_maps.append(m)
    return in_maps


def kernel(input, freqs_cos, freqs_sin, qw_w, qw_b, kw_w, kw_b, vw_w, vw_b,
           ow_w, ow_b, start_pos):
    in_maps = _make_in_maps(dict(
        input=input, freqs_cos=freqs_cos, freqs_sin=freqs_sin,
        qw_w=qw_w, qw_b=qw_b, kw_w=kw_w, kw_b=kw_b, vw_w=vw_w, vw_b=vw_b,
        ow_w=ow_w, ow_b=ow_b,
    ))
    nc = _get_nc()
    res = run_bass_kernel_spmd(nc, in_maps, list(range(NCORES)))

    out = np.empty((B, S, D), np.float32)
    for core in range(NCORES):
        b, chunks = _core_chunks(core)
        r = res.results[core]["out"].reshape(2, CH, D)
        for sl in range(2):
            c0 = chunks[sl] * CH
            out[b, c0 : c0 + CH, :] = r[sl]
    return out



# revision 12
# speedup vs baseline: 1.3957x; 1.3957x over previous
"""GQA attention prefill (B=4, S=2048, D=4096, 32 q-heads / 8 kv-heads, rotary,
causal) on 8 TRN2 NeuronCores.

Sharding: 2-way tensor-parallel (kv-head groups, 16 q-heads / 4 kv-heads per
half) x 4-way data-parallel (batch). Core c = (batch c//2, half c%2). Each
core computes Q/K/V projections for its heads over its batch's full sequence,
attention, and a PARTIAL output projection (its heads' input columns of ow);
the host sums the two partials per batch. No inter-core communication.

Precision plan (validated against the reference on CPU):
  - Q/K/V projections: 3-term fp8-e4m3 DoubleRow matmuls
    (xh*(Wh+Wl) + xl*Wh with x = xh+xl, W*64 = Wh+Wl), f32 PSUM, descaled
    by 1/64 at eviction. DoubleRow costs 0.5 cyc/row with 256-deep
    contraction -> 1.5 cyc/row total vs bf16's 2.
  - rotary: PSUM -> bf16 copy (scalar engine, scale 1/64), then 6 DVE bf16
    ops (4x mode) per 64x512 evict; bias folded into the scalar operand.
  - attention: scores-transposed [key, query] orientation; QK / exp /
    AV / ones-denominator all bf16 (fp8 here fails the 2e-2 tolerance:
    logit noise and v-quantization pass ~1:1 through peaked softmax rows).
  - attention output: normalized on DVE, split into fp8 hi+lo, feeding a
    3-term fp8 DoubleRow output projection (ow split hi/lo host-side).
  - biases (all zero in this problem, handled generally): Q/K in the rotary
    scalar op; V/O as K=1 ones-outer-product matmuls into PSUM.

Schedule: phase P per chunk streams x + qw from DRAM, runs V/K/Q
projections (qT spilled to DRAM to bound SBUF); phase A runs attention per
chunk with the previous chunk's output projection woven into the PE stream
as a feeder (attention alone is scalar-engine-bound; o-proj fills the PE).
"""

import numpy as np
import ml_dtypes

import concourse.bacc as bacc
import concourse.bass as bass
import concourse.tile as tile
from concourse import library_config, mybir
from concourse.bass_utils import run_bass_kernel_spmd

F32 = mybir.dt.float32
F32R = mybir.dt.float32r
BF16 = mybir.dt.bfloat16
F8 = mybir.dt.float8e4
DR = mybir.MatmulPerfMode.DoubleRow
EXP = mybir.ActivationFunctionType.Exp
COPY = mybir.ActivationFunctionType.Copy
ADD = mybir.AluOpType.add
MULT = mybir.AluOpType.mult

B, S, D = 4, 2048, 4096
QH, KVH, HEAD = 32, 8, 128
P = 128
CH = 512                 # token chunk (= query tile = moving free dim)
NCH = S // CH            # 4 chunks
NKT = 16                 # 256-deep contraction tiles over D
TPH = 16                 # q heads per TP half
KVL = 4                  # kv heads per TP half
NCORES = 8
SCALE = 1.0 / np.sqrt(HEAD)
BF = ml_dtypes.bfloat16
F8NP = ml_dtypes.float8_e4m3

_CACHE = {}


def _build():
    nc = bacc.Bacc("TRN2", target_bir_lowering=False, debug=False, num_devices=NCORES)

    # ---- per-core external inputs ----
    # x / weights in DoubleRow contraction layout: element (p, kt, s) is
    # input dim d = kt*256 + s*128 + p.
    x8 = nc.dram_tensor("x8", [NCH, P, 2, NKT, 2, CH], F8, kind="ExternalInput")
    qw8 = nc.dram_tensor("qw8", [TPH, P, 2, NKT, 2, P], F8, kind="ExternalInput")
    kw8 = nc.dram_tensor("kw8", [KVL, P, 2, NKT, 2, P], F8, kind="ExternalInput")
    vw8 = nc.dram_tensor("vw8", [P, 2, NKT, 2, KVL * P], F8, kind="ExternalInput")
    ow8 = nc.dram_tensor("ow8", [8, P, 2, 8, 2, CH], F8, kind="ExternalInput")
    cos16 = nc.dram_tensor("cos16", [64, S], BF16, kind="ExternalInput")
    sin16 = nc.dram_tensor("sin16", [64, S], BF16, kind="ExternalInput")
    qb = nc.dram_tensor("qb", [64, 2, TPH], F32, kind="ExternalInput")
    kb = nc.dram_tensor("kb", [64, 2, KVL], F32, kind="ExternalInput")
    vb16 = nc.dram_tensor("vb16", [1, KVL * P], BF16, kind="ExternalInput")
    ob16 = nc.dram_tensor("ob16", [1, D], BF16, kind="ExternalInput")
    ones_row = nc.dram_tensor("ones_row", [1, P], BF16, kind="ExternalInput")
    ones_col = nc.dram_tensor("ones_col", [P, 1], BF16, kind="ExternalInput")
    mask16 = nc.dram_tensor("mask16", [P, 4, CH], BF16, kind="ExternalInput")

    # qT spill (bf16, deinterleaved rotary dims on partitions)
    qT_i = nc.dram_tensor("qT_i", [NCH, TPH, P, CH], BF16)

    out = nc.dram_tensor("out", [16, P, 8, CH], F32, kind="ExternalOutput")

    with tile.TileContext(nc) as tc:
        nc.gpsimd.load_library(library_config.lib)
        with tc.tile_pool(name="const", bufs=1) as const:
            cos_sb = const.tile([64, S], BF16, tag="cos")
            sin_sb = const.tile([64, S], BF16, tag="sin")
            nc.sync.dma_start(out=cos_sb[:], in_=cos16[:])
            nc.sync.dma_start(out=sin_sb[:], in_=sin16[:])
            qb_sb = const.tile([64, 2, TPH], F32, tag="qb")
            kb_sb = const.tile([64, 2, KVL], F32, tag="kb")
            nc.sync.dma_start(out=qb_sb[:], in_=qb[:])
            nc.sync.dma_start(out=kb_sb[:], in_=kb[:])
            vb_sb = const.tile([1, KVL * P], BF16, tag="vb")
            nc.sync.dma_start(out=vb_sb[:], in_=vb16[:])
            onr_sb = const.tile([1, P], BF16, tag="onr")
            onc_sb = const.tile([P, 1], BF16, tag="onc")
            nc.sync.dma_start(out=onr_sb[:], in_=ones_row[:])
            nc.sync.dma_start(out=onc_sb[:], in_=ones_col[:])

            # persistent across phases
            kT = const.tile([P, KVL, S], BF16, tag="kT")
            v16 = const.tile([P, NCH * 4, KVL * P], BF16, tag="v16")

            def proj3(ps, xt, wt, nkt):
                """3-term fp8 DR accumulation: xh*(Wh+Wl) + xl*Wh."""
                n = 0
                tot = 3 * nkt
                for kt in range(nkt):
                    for xi, wi in ((0, 0), (0, 1), (1, 0)):
                        nc.tensor.matmul(
                            ps[:], lhsT=wt[:, wi, kt, :, :], rhs=xt[:, xi, kt, :, :],
                            start=(n == 0), stop=(n == tot - 1), perf_mode=DR,
                        )
                        n += 1

            def rotary(ps, dst_r, dst_i, be, bo, cs, sn, rpool):
                """dst_r = (pe+be)*cos - (po+bo)*sin; dst_i = (pe+be)*sin + (po+bo)*cos.
                ps is 64-scaled; cos/sin tables are pre-divided by 64 and biases
                pre-multiplied by 64 host-side. stt reads PSUM directly (PSUM+SB
                operand mix is exempt from the same-base-partition rule)."""
                pe, po = ps[0:64, :], ps[64:P, :]
                t1 = rpool.tile([64, CH], BF16, tag="t1", name="t1")
                t2 = rpool.tile([64, CH], BF16, tag="t2", name="t2")
                t3 = rpool.tile([64, CH], BF16, tag="t3", name="t3")
                t4 = rpool.tile([64, CH], BF16, tag="t4", name="t4")
                nc.vector.scalar_tensor_tensor(t1[:], pe, be, cs, ADD, MULT)
                nc.vector.scalar_tensor_tensor(t2[:], po, bo, sn, ADD, MULT)
                nc.vector.scalar_tensor_tensor(t3[:], pe, be, sn, ADD, MULT)
                nc.vector.scalar_tensor_tensor(t4[:], po, bo, cs, ADD, MULT)
                nc.vector.tensor_sub(dst_r, t1[:], t2[:])
                nc.vector.tensor_add(dst_i, t3[:], t4[:])

            # ================= Phase P: projections =================
            with (
                tc.tile_pool(name="xp", bufs=2) as xpool,
                tc.tile_pool(name="wst", bufs=2) as wstream,
                tc.tile_pool(name="wres", bufs=1) as wres,
                tc.tile_pool(name="rt", bufs=2) as rpool,
                tc.tile_pool(name="qtw", bufs=2) as qtwpool,
                tc.tile_pool(name="pp", bufs=4, space="PSUM") as pp,
            ):
                kwt = wres.tile([P, KVL, 2, NKT, 2, P], F8, tag="kw")
                nc.sync.dma_start(
                    out=kwt[:], in_=kw8.rearrange("k p a b c d -> p k a b c d")
                )
                vwt = wres.tile([P, 2, NKT, 2, KVL * P], F8, tag="vw")
                nc.sync.dma_start(out=vwt[:], in_=vw8[:])

                for c in range(NCH):
                    xt = xpool.tile([P, 2, NKT, 2, CH], F8, tag="x", name=f"x{c}")
                    nc.sync.dma_start(out=xt[:], in_=x8[c])
                    # --- V proj (input-stationary) ---
                    for ts in range(4):
                        ps = pp.tile([P, CH], F32, tag="ps", name="psv")
                        nc.tensor.matmul(
                            ps[:], lhsT=onr_sb[:], rhs=vb_sb[:],
                            start=True, stop=False,
                        )
                        n, tot = 0, 3 * NKT
                        for kt in range(NKT):
                            for xi, wi in ((0, 0), (0, 1), (1, 0)):
                                nc.tensor.matmul(
                                    ps[:],
                                    lhsT=xt[:, xi, kt, :, ts * P:(ts + 1) * P],
                                    rhs=vwt[:, wi, kt, :, :],
                                    start=False, stop=(n == tot - 1), perf_mode=DR,
                                )
                                n += 1
                        nc.scalar.activation(
                            v16[:, 4 * c + ts, :], ps[:], COPY, scale=1.0 / 64.0
                        )
                    # --- K proj ---
                    for kv in range(KVL):
                        ps = pp.tile([P, CH], F32, tag="ps", name="psk")
                        proj3(ps, xt, kwt[:, kv], NKT)
                        rotary(
                            ps, kT[0:64, kv, c * CH:(c + 1) * CH],
                            kT[64:P, kv, c * CH:(c + 1) * CH],
                            kb_sb[:, 0, kv:kv + 1], kb_sb[:, 1, kv:kv + 1],
                            cos_sb[:, c * CH:(c + 1) * CH],
                            sin_sb[:, c * CH:(c + 1) * CH], rpool,
                        )
                    # --- Q proj (weights streamed) ---
                    for hb in range(TPH):
                        wt = wstream.tile([P, 2, NKT, 2, P], F8, tag="qw", name="qw")
                        nc.gpsimd.dma_start(out=wt[:], in_=qw8[hb])
                        ps = pp.tile([P, CH], F32, tag="ps", name="psq")
                        proj3(ps, xt, wt, NKT)
                        qt = qtwpool.tile([P, CH], BF16, tag="qt", name="qt")
                        rotary(
                            ps, qt[0:64, :], qt[64:P, :],
                            qb_sb[:, 0, hb:hb + 1], qb_sb[:, 1, hb:hb + 1],
                            cos_sb[:, c * CH:(c + 1) * CH],
                            sin_sb[:, c * CH:(c + 1) * CH], rpool,
                        )
                        nc.sync.dma_start(out=qT_i[c, hb], in_=qt[:])

            # ================= Phase A: attention + woven o-proj =================
            with (
                tc.tile_pool(name="aconst", bufs=1) as aconst,
                tc.tile_pool(name="qts", bufs=2) as qtspool,
                tc.tile_pool(name="pt", bufs=4) as ptpool,
                tc.tile_pool(name="fin", bufs=2) as finpool,
                tc.tile_pool(name="osb", bufs=2) as osbpool,
                tc.tile_pool(name="owst", bufs=2) as owstream,
                tc.tile_pool(name="oout", bufs=3) as ooutpool,
                tc.tile_pool(name="psc", bufs=3, space="PSUM") as pscpool,
                tc.tile_pool(name="pav", bufs=2, space="PSUM") as pavpool,
                tc.tile_pool(name="pdn", bufs=1, space="PSUM") as pdnpool,
                tc.tile_pool(name="pox", bufs=2, space="PSUM") as poxpool,
            ):
                ob_sb = aconst.tile([1, D], BF16, tag="ob")
                nc.sync.dma_start(out=ob_sb[:], in_=ob16[:])
                mask_sb = aconst.tile([P, 4, CH], BF16, tag="mask")
                nc.sync.dma_start(out=mask_sb[:], in_=mask16[:])
                def oproj(c):
                    """O-proj for chunk c (generator; yields after each matmul)."""
                    osh, osl = osb_tiles[c]
                    for ob in range(8):
                        owt = owstream.tile([P, 2, 8, 2, CH], F8, tag="ow", name="ow")
                        nc.gpsimd.dma_start(out=owt[:], in_=ow8[ob])
                        for ts in range(4):
                            po = poxpool.tile([P, CH], F32, tag="po", name="po")
                            nc.tensor.matmul(
                                po[:], lhsT=onr_sb[:],
                                rhs=ob_sb[:, ob * CH:(ob + 1) * CH],
                                start=True, stop=False,
                            )
                            yield
                            n, tot = 0, 24
                            for hp in range(8):
                                for ai, wi in ((0, 0), (0, 1), (1, 0)):
                                    a = osh if ai == 0 else osl
                                    nc.tensor.matmul(
                                        po[:],
                                        lhsT=a[:, hp, :, ts * P:(ts + 1) * P],
                                        rhs=owt[:, wi, hp, :, :],
                                        start=False, stop=(n == tot - 1),
                                        perf_mode=DR,
                                    )
                                    n += 1
                                    yield
                            oo = ooutpool.tile([P, CH], F32, tag="oo", name="oo")
                            nc.scalar.activation(oo[:], po[:], COPY, scale=1.0 / 64.0)
                            nc.gpsimd.dma_start(
                                out=out[4 * c + ts, :, ob, :], in_=oo[:]
                            )

                osb_tiles = {}
                feeder = None
                for c in range(NCH):
                    osh = osbpool.tile([P, 8, 2, CH], F8, tag="osh", name=f"osh{c}")
                    osl = osbpool.tile([P, 8, 2, CH], F8, tag="osl", name=f"osl{c}")
                    osb_tiles[c] = (osh, osl)
                    qts = qtspool.tile([P, TPH, CH], BF16, tag="qts", name="qts")
                    nc.sync.dma_start(
                        out=qts[:], in_=qT_i[c].rearrange("h p t -> p h t")
                    )
                    nb = 4 * (c + 1)
                    pull = {0: 0, 1: 7, 2: 4, 3: 3}[c]
                    for hl in range(TPH):
                        kvl = hl // 4
                        pav = pavpool.tile([P, CH], F32, tag="pav", name="pav")
                        pdn = pdnpool.tile([P, CH], F32, tag="pdn", name="pdn")
                        pts = [None] * nb

                        def do_av(kb):
                            nc.tensor.matmul(
                                pav[:], lhsT=v16[:, kb, kvl * P:(kvl + 1) * P],
                                rhs=pts[kb][:], start=(kb == 0), stop=(kb == nb - 1),
                            )
                            nc.tensor.matmul(
                                pdn[0:1, :], lhsT=onc_sb[:], rhs=pts[kb][:],
                                start=(kb == 0), stop=(kb == nb - 1),
                            )

                        for kb in range(nb):
                            psc = pscpool.tile([P, CH], F32, tag="psc", name="psc")
                            nc.tensor.matmul(
                                psc[:], lhsT=kT[:, kvl, kb * P:(kb + 1) * P],
                                rhs=qts[:, hl, :], start=True, stop=True,
                            )
                            pt = ptpool.tile([P, CH], BF16, tag="pt", name="pt")
                            pts[kb] = pt
                            nc.scalar.activation(pt[:], psc[:], EXP, scale=SCALE)
                            j = kb - (nb - 4)
                            if j >= 0:
                                w = P * (j + 1)
                                nc.vector.tensor_mul(
                                    pt[:, 0:w], pt[:, 0:w], mask_sb[:, j, 0:w]
                                )
                            if kb >= 2:
                                do_av(kb - 2)
                            if feeder is not None:
                                for _ in range(pull):
                                    if next(feeder, "END") == "END":
                                        feeder = None
                                        break
                        do_av(nb - 2)
                        do_av(nb - 1)
                        # normalize + fp8 hi/lo split
                        rsb = finpool.tile([1, CH], F32R, tag="rsb", name="rsb")
                        with nc.allow_low_precision(reason="softmax denom"):
                            nc.vector.reciprocal(rsb[:], pdn[0:1, :])
                        rbc = finpool.tile([P, CH], F32R, tag="rbc", name="rbc")
                        nc.gpsimd.partition_broadcast(rbc[:], rsb[:])
                        t = finpool.tile([P, CH], F32, tag="t", name="t")
                        nc.vector.tensor_mul(t[:], pav[:], rbc[:])
                        hp, sl = hl // 2, hl % 2
                        nc.scalar.activation(osh[:, hp, sl, :], t[:], COPY)
                        nc.vector.tensor_sub(osl[:, hp, sl, :], t[:], osh[:, hp, sl, :])
                    if feeder is not None:
                        for _ in feeder:
                            pass
                    feeder = oproj(c)
                for _ in feeder:
                    pass

    nc.compile()
    return nc


def _get_nc():
    if "nc" not in _CACHE:
        _CACHE["nc"] = _build()
    return _CACHE["nc"]


_PERM = np.concatenate([np.arange(0, P, 2), np.arange(1, P, 2)])


def _hilo(a):
    hi = a.astype(F8NP)
    lo = (a - hi.astype(np.float32)).astype(F8NP)
    return hi, lo


def _dr_layout(a):
    """[n, 4096] -> [128 p, 16 kt, 2 s, n]: element (p,kt,s) = dim kt*256+s*128+p."""
    n = a.shape[0]
    return np.ascontiguousarray(
        a.reshape(n, NKT, 2, P).transpose(3, 1, 2, 0)
    )


def _make_in_maps(inputs):
    g = lambda k: np.asarray(inputs[k]).astype(np.float32)
    qw, qwb = g("qw_w"), g("qw_b")
    kw, kwb = g("kw_w"), g("kw_b")
    vw, vwb = g("vw_w"), g("vw_b")
    ow, owb = g("ow_w"), g("ow_b")
    fc, fs = g("freqs_cos"), g("freqs_sin")
    x = g("input")

    cos16 = np.ascontiguousarray((fc.T / 64.0).astype(BF))
    sin16 = np.ascontiguousarray((fs.T / 64.0).astype(BF))
    masks = np.zeros((P, 4, CH), BF)
    kp = np.arange(P)[:, None]
    qi = np.arange(CH)[None, :]
    for j in range(4):
        masks[:, j, :] = (P * j + kp <= qi).astype(BF)

    halves = []
    for t in range(2):
        kvs = [4 * t + j for j in range(4)]
        qhs = [g_ * 8 + kv for kv in kvs for g_ in range(4)]  # hl = kvl*4 + g
        rq = np.concatenate([np.arange(H * P, (H + 1) * P) for H in qhs])
        rk = np.concatenate([np.arange(kv * P, (kv + 1) * P) for kv in kvs])
        rq_perm = np.concatenate([H * P + _PERM for H in qhs])
        rk_perm = np.concatenate([kv * P + _PERM for kv in kvs])

        # Q/K weights: rows permuted (rotary deinterleave), x64, hi/lo, DR layout
        def wprep(Wsel, nh):
            Wh, Wl = _hilo(64.0 * Wsel)
            out = np.empty((nh, P, 2, NKT, 2, P), F8NP)
            for h in range(nh):
                for i, W in enumerate((Wh, Wl)):
                    blk = W[h * P:(h + 1) * P]          # [128 rows, 4096]
                    out[h, :, i] = _dr_layout(blk)       # [p, kt, s, 128 rows]
            return np.ascontiguousarray(out)

        qw8 = wprep(qw[rq_perm], TPH)
        kw8 = wprep(kw[rk_perm], KVL)

        vh, vl = _hilo(64.0 * vw[rk])                    # [512 rows, 4096]
        vw8 = np.stack([_dr_layout(vh), _dr_layout(vl)], axis=1)
        vw8 = np.ascontiguousarray(vw8)                  # [p, 2, kt, s, 512]

        # ow: [8 ob][128 hd][2 term][8 hp][2 hslot][512 od]
        Woh, Wol = _hilo(64.0 * ow[:, rq])               # [4096 od, 2048 col]
        ow8 = np.empty((8, P, 2, 8, 2, CH), F8NP)
        for i, W in enumerate((Woh, Wol)):
            wr = W.reshape(8, CH, TPH, P)                # [ob, od, hl, hd]
            ow8[:, :, i] = wr.transpose(0, 3, 2, 1).reshape(8, P, 8, 2, CH)
        ow8 = np.ascontiguousarray(ow8)

        qb_l = np.ascontiguousarray(
            (64.0 * qwb[rq_perm]).reshape(TPH, 2, 64).transpose(2, 1, 0).astype(np.float32))
        kb_l = np.ascontiguousarray(
            (64.0 * kwb[rk_perm]).reshape(KVL, 2, 64).transpose(2, 1, 0).astype(np.float32))
        vb_l = np.ascontiguousarray((64.0 * vwb[rk])[None, :].astype(BF))
        ob_l = np.ascontiguousarray((64.0 * owb)[None, :].astype(BF))
        halves.append(dict(
            qw8=qw8, kw8=kw8, vw8=vw8, ow8=ow8, qb=qb_l, kb=kb_l,
            vb16=vb_l, ob16=ob_l,
        ))

    shared = dict(
        cos16=cos16, sin16=sin16, mask16=masks,
        ones_row=np.ones((1, P), BF), ones_col=np.ones((P, 1), BF),
    )
    in_maps = []
    for core in range(NCORES):
        b, t = core // 2, core % 2
        xh, xl = _hilo(x[b])                             # [2048, 4096]
        x8 = np.empty((NCH, P, 2, NKT, 2, CH), F8NP)
        for c in range(NCH):
            x8[c, :, 0] = _dr_layout(xh[c * CH:(c + 1) * CH])
            x8[c, :, 1] = _dr_layout(xl[c * CH:(c + 1) * CH])
        m = dict(shared)
        m.update(halves[t])
        m["x8"] = np.ascontiguousarray(x8)
        in_maps.append(m)
    return in_maps


def kernel(input, freqs_cos, freqs_sin, qw_w, qw_b, kw_w, kw_b, vw_w, vw_b,
           ow_w, ow_b, start_pos):
    in_maps = _make_in_maps(dict(
        input=input, freqs_cos=freqs_cos, freqs_sin=freqs_sin,
        qw_w=qw_w, qw_b=qw_b, kw_w=kw_w, kw_b=kw_b, vw_w=vw_w, vw_b=vw_b,
        ow_w=ow_w, ow_b=ow_b,
    ))
    nc = _get_nc()
    res = run_bass_kernel_spmd(nc, in_maps, list(range(NCORES)))

    out = np.zeros((B, S, D), np.float32)
    for core in range(NCORES):
        b = core // 2
        r = res.results[core]["out"]                     # [16, 128, 8, 512]
        part = r.transpose(0, 1, 2, 3).reshape(16 * P, D)
        out[b] += part.reshape(S, D)
    return out
